# revision 13
# baseline (speedup 1.0000x reference)
"""Trainium2 Bass kernel for nn_Dist_Conv2D_Dense (Chebyshev-distance "conv").

Computation (per batch b, output channel co, position (h, w)):
    out[b, co, h, w] = max_{cin, kh, kw} |x[b, cin, h+kh-1, w+kw-1] - weights[co, cin, kh, kw]| + bias[co]
with replicate ("edge") padding, for x (8, 16, 64, 64), weights (32, 16, 3, 3).

Sharding: data-parallel over batch, B=8 -> one batch element per NeuronCore.

SCHEME "pe" (default, fastest):
  The 151M (x - w) differences are produced by the TensorEngine as a
  "selector" matmul: stationary lhsT = 73 rows holding the 72 shifted input
  planes of one half of the 144-long window (plus a constant-ones row), moving
  rhs = selector columns with a single 1 at row d and -w[co, d] in the ones
  row.  psum[p, (co, j)] = x_win[p, d] - w[co, d], 128 output positions
  (an h-row-pair x 64 columns) per column.  Consumers split per 16-unit chunk:
    - direct:  DVE tensor_reduce(max, abs) straight from PSUM
    - staged:  ScalarE Abs-activation drains PSUM -> fp16 SBUF, then a DVE
               fp16 tensor_tensor(max) tree (2x mode) + small reduce tail.
  Halves are combined with one tensor_tensor(max), bias added, DMA'd out
  contiguously, and unshuffled to (B, Cout, H, W) on host.

SCHEME "dve": simpler 2-instruction-per-row-pair DVE-only variant
  (broadcast subtract + abs-max reduce).
"""

import numpy as np
from contextlib import ExitStack

# Problem constants (hardcoded per spec)
B, CIN, H, W = 8, 16, 64, 64
COUT, K = 32, 3
N_CORES = 8
HPAD = H + 2  # 66
D = CIN * K * K  # 144
DH = D // 2  # 72, half-window length

SCHEME = "hybrid"  # "hybrid" | "pe" | "dve"
COMPUTE = "f16"  # dtype for the DVE scheme ("f32" | "f16")
DIRECT_CHUNKS = 1  # "pe" scheme: how many of the 4 chunks/row-pair DVE reduces
                   # directly from PSUM (the rest go ScalarE-abs -> fp16 tree)
# "hybrid" scheme channel split: first NB channels' diffs on TensorE,
# next NC = 32-NB-ND on VectorE subtract + ScalarE abs, last ND on VectorE
# subtract + VectorE int16-mask abs.
NB = 24
ND = 4

_PROGRAM_CACHE = {}
LAST_RESULTS = None  # stashed BassKernelResults for the test harness


# ------------------------------------------------------------ hybrid scheme

def _build_program_hybrid():
    import concourse.bacc as bacc
    import concourse.mybir as mybir
    from concourse.alu_op_type import AluOpType
    from concourse.tile import TileContext

    F16, F32, I16 = mybir.dt.float16, mybir.dt.float32, mybir.dt.int16
    NC = 32 - NB - ND          # DVE-sub + ACT-abs channels
    NCD = NC + ND              # all DVE-subtracted channels
    PCOLS = NB * DH            # psum columns per half-chunk

    nc = bacc.Bacc(
        "TRN2", target_bir_lowering=False, debug=False, num_devices=N_CORES
    )

    xa_d = nc.dram_tensor("xa", [DH + 1, H * W], F16, kind="ExternalInput")
    xb_d = nc.dram_tensor("xb", [DH + 1, H * W], F16, kind="ExternalInput")
    sa_d = nc.dram_tensor("sa", [DH + 1, PCOLS], F16, kind="ExternalInput")
    sb_d = nc.dram_tensor("sb", [DH + 1, PCOLS], F16, kind="ExternalInput")
    x3b_d = nc.dram_tensor("x3b", [128, 3 * HPAD * CIN], F16, kind="ExternalInput")
    wcd_d = nc.dram_tensor("wcd", [128, NCD * D], F16, kind="ExternalInput")
    bias_d = nc.dram_tensor("biasb", [128, 32 * COUT], F16, kind="ExternalInput")
    out_d = nc.dram_tensor("out", [128, 32 * COUT], F32, kind="ExternalOutput")

    with TileContext(nc) as tc:
        with (
            tc.tile_pool(name="io", bufs=1) as io_pool,
            tc.tile_pool(name="ps", bufs=2, space="PSUM") as ps_pool,
            tc.tile_pool(name="st", bufs=2) as st_pool,
            tc.tile_pool(name="sc", bufs=2) as sc_pool,
            tc.tile_pool(name="tr", bufs=2) as tr_pool,
        ):
            # spread the input loads across per-engine DMA queues
            xa_t = io_pool.tile([DH + 1, H * W], F16)
            nc.scalar.dma_start(out=xa_t[:, :], in_=xa_d.ap())
            xb_t = io_pool.tile([DH + 1, H * W], F16)
            nc.gpsimd.dma_start(out=xb_t[:, :], in_=xb_d.ap())
            sa_t = io_pool.tile([DH + 1, PCOLS], F16)
            nc.sync.dma_start(out=sa_t[:, :], in_=sa_d.ap())
            sb_t = io_pool.tile([DH + 1, PCOLS], F16)
            nc.scalar.dma_start(out=sb_t[:, :], in_=sb_d.ap())
            x3b_t = io_pool.tile([128, 3 * HPAD * CIN], F16)
            nc.sync.dma_start(out=x3b_t[:, :], in_=x3b_d.ap())
            wcd_t = io_pool.tile([128, NCD * D], F16)
            nc.gpsimd.dma_start(out=wcd_t[:, :], in_=wcd_d.ap())
            bias_t = io_pool.tile([128, 32 * COUT], F16)
            nc.scalar.dma_start(out=bias_t[:, :], in_=bias_d.ap())

            # acc columns: (r2, unit) with 64 units of 72:
            #   units 0..NB-1        : PE channels, low half (d < 72)
            #   units NB..2NB-1      : PE channels, high half
            #   units 2NB..2NB+2NCD-1: DVE channels, (co, half) interleaved
            acc_t = io_pool.tile([128, 32 * 64], F32)
            fin_t = io_pool.tile([128, 32 * COUT], F32)

            xa4 = xa_t[:, :].rearrange("k (h w) -> k h w", h=H)
            xb4 = xb_t[:, :].rearrange("k (h w) -> k h w", h=H)
            x4 = x3b_t[:, :].rearrange("p (kw h c) -> p kw h c", kw=3, h=HPAD)
            w5 = wcd_t[:, :].rearrange(
                "p (co kw kh c) -> p co kw kh c", co=NCD, kw=3, kh=3
            )

            def emit_tree(r2, staged):
                # VectorE fp16 max tree over all 64 units of 72
                s3 = staged[:, :].rearrange("p (u j) -> p u j", j=DH)
                t1_t = tr_pool.tile([128, 64 * 36], F16, tag="t1", name=f"t1_{r2}")
                t1 = t1_t[:, :].rearrange("p (u j) -> p u j", j=36)
                nc.vector.tensor_tensor(
                    out=t1, in0=s3[:, :, 0:36], in1=s3[:, :, 36:72], op=AluOpType.max
                )
                t2_t = tr_pool.tile([128, 64 * 18], F16, tag="t2", name=f"t2_{r2}")
                t2 = t2_t[:, :].rearrange("p (u j) -> p u j", j=18)
                nc.vector.tensor_tensor(
                    out=t2, in0=t1[:, :, 0:18], in1=t1[:, :, 18:36], op=AluOpType.max
                )
                t3_t = tr_pool.tile([128, 64 * 9], F16, tag="t3", name=f"t3_{r2}")
                t3 = t3_t[:, :].rearrange("p (u j) -> p u j", j=9)
                nc.vector.tensor_tensor(
                    out=t3, in0=t2[:, :, 0:9], in1=t2[:, :, 9:18], op=AluOpType.max
                )
                nc.vector.tensor_reduce(
                    out=acc_t[:, r2 * 64 : (r2 + 1) * 64],
                    in_=t3,
                    axis=mybir.AxisListType.X,
                    op=AluOpType.max,
                )

            pending = None  # (r2, staged) whose tree is not yet emitted
            for r2 in range(32):
                r = 2 * r2
                staged = st_pool.tile([128, 64 * DH], F16, tag="stg", name=f"stg{r2}")

                # --- TensorE: diffs for the first NB channels, two halves ---
                for half in range(2):
                    x_t = (xa4 if half == 0 else xb4)[:, r : r + 2, :]  # [73,2,64]
                    s_t = sa_t if half == 0 else sb_t
                    ps_t = ps_pool.tile([128, PCOLS], F32, tag="ps", name=f"ps{r2}_{half}")
                    for m0 in range(0, PCOLS, 512):
                        m1 = min(m0 + 512, PCOLS)
                        nc.tensor.matmul(
                            out=ps_t[:, m0:m1],
                            lhsT=x_t,
                            rhs=s_t[:, m0:m1],
                            start=True,
                            stop=True,
                        )
                    # ScalarE drains + abs + casts to fp16
                    nc.scalar.activation(
                        out=staged[:, half * PCOLS : (half + 1) * PCOLS],
                        in_=ps_t[:, :],
                        func=mybir.ActivationFunctionType.Abs,
                    )

                # --- VectorE subtract for the last NCD channels ---
                sc_t = sc_pool.tile([128, NCD * D], F16, tag="sc", name=f"sc{r2}")
                s5 = sc_t[:, :].rearrange(
                    "p (co kw kh c) -> p co kw kh c", co=NCD, kw=3, kh=3
                )
                x5b = (
                    x4[:, :, r : r + 3, :]
                    .unsqueeze(1)
                    .broadcast_to((128, NCD, 3, 3, CIN))
                )
                nc.vector.tensor_tensor(out=s5, in0=x5b, in1=w5, op=AluOpType.subtract)
                # ScalarE abs for the NC channels
                if NC:
                    nc.scalar.activation(
                        out=staged[:, 2 * PCOLS : 2 * PCOLS + NC * D],
                        in_=sc_t[:, 0 : NC * D],
                        func=mybir.ActivationFunctionType.Abs,
                    )
                # VectorE int16 sign-mask abs for the ND channels
                if ND:
                    nc.vector.tensor_scalar(
                        out=staged[:, 2 * PCOLS + NC * D : 64 * DH].bitcast(I16),
                        in0=sc_t[:, NC * D : NCD * D].bitcast(I16),
                        scalar1=0x7FFF,
                        scalar2=None,
                        op0=AluOpType.bitwise_and,
                    )

                # software pipeline: emit the previous row-pair's tree now, so
                # VectorE trees r2-1 while ScalarE is still absing r2
                if pending is not None:
                    emit_tree(*pending)
                pending = (r2, staged)

            emit_tree(*pending)

            # --- combine halves + bias ---
            a4 = acc_t[:, :].rearrange("p (r2 u) -> p r2 u", u=64)
            f3 = fin_t[:, :].rearrange("p (r2 c) -> p r2 c", c=COUT)
            # PE channels: units u and u+NB
            nc.vector.tensor_tensor(
                out=f3[:, :, 0:NB],
                in0=a4[:, :, 0:NB],
                in1=a4[:, :, NB : 2 * NB],
                op=AluOpType.max,
            )
            # DVE channels: interleaved halves
            acd = acc_t[:, :].rearrange(
                "p (r2 u) -> p r2 u", u=64
            )[:, :, 2 * NB : 64].rearrange("p r2 (c h) -> p r2 c h", h=2)
            nc.vector.tensor_tensor(
                out=f3[:, :, NB:COUT],
                in0=acd[:, :, :, 0],
                in1=acd[:, :, :, 1],
                op=AluOpType.max,
            )
            nc.vector.tensor_tensor(
                out=fin_t[:, :], in0=fin_t[:, :], in1=bias_t[:, :], op=AluOpType.add
            )
            nc.sync.dma_start(out=out_d.ap(), in_=fin_t[:, :])

    nc.compile()
    return nc


def _prep_inputs_hybrid(x, weights, bias):
    NC = 32 - NB - ND
    NCD = NC + ND
    PCOLS = NB * DH
    w_perm = np.ascontiguousarray(weights.transpose(0, 3, 2, 1)).reshape(COUT, D)

    def selector(half):
        s = np.zeros((DH + 1, NB, DH), dtype=np.float32)
        for j in range(DH):
            s[j, :, j] = 1.0
        s[DH, :, :] = -w_perm[:NB, half * DH : (half + 1) * DH]
        return np.ascontiguousarray(s.reshape(DH + 1, PCOLS).astype(np.float16))

    sa = selector(0)
    sb = selector(1)
    wcd = np.ascontiguousarray(
        np.broadcast_to(
            w_perm[NB:].reshape(1, NCD * D), (128, NCD * D)
        ).astype(np.float16)
    )
    biasb = np.ascontiguousarray(
        np.broadcast_to(
            np.tile(bias.reshape(COUT), 32)[None, :], (128, 32 * COUT)
        ).astype(np.float16)
    )

    in_maps = []
    for core in range(N_CORES):
        xc = x[core]
        x_pad = np.pad(xc, ((0, 0), (1, 1), (1, 1)), mode="edge")
        planes = np.empty((3, 3, CIN, H, W), dtype=np.float32)  # (kw, kh, cin, h, w)
        for kw in range(3):
            for kh in range(3):
                planes[kw, kh] = x_pad[:, kh : kh + H, kw : kw + W]
        planes = planes.reshape(D, H * W)
        ones = np.ones((1, H * W), dtype=np.float32)
        xa = np.concatenate([planes[:DH], ones], axis=0).astype(np.float16)
        xb = np.concatenate([planes[DH:], ones], axis=0).astype(np.float16)
        in_maps.append(
            {
                "xa": np.ascontiguousarray(xa),
                "xb": np.ascontiguousarray(xb),
                "sa": sa,
                "sb": sb,
                "x3b": _build_x3b_f16(xc),
                "wcd": wcd,
                "biasb": biasb,
            }
        )
    return in_maps


def _build_x3b_f16(xc):
    wi = np.clip(np.arange(W)[None, :] + np.arange(-1, 2)[:, None], 0, W - 1)
    halves = []
    for b in range(2):
        h_idx = np.clip(np.arange(HPAD) - 1 + b, 0, H - 1)
        g = xc[:, h_idx, :][:, :, wi]  # (CIN, HPAD, 3, W)
        halves.append(np.ascontiguousarray(g.transpose(3, 2, 1, 0)))
    out = np.stack(halves, axis=0)  # (2, W, 3, HPAD, CIN)
    return np.ascontiguousarray(out.reshape(128, 3 * HPAD * CIN).astype(np.float16))


# ---------------------------------------------------------------- PE scheme

def _build_program_pe():
    import concourse.bacc as bacc
    import concourse.mybir as mybir
    from concourse.alu_op_type import AluOpType
    from concourse.tile import TileContext

    F16, F32 = mybir.dt.float16, mybir.dt.float32

    nc = bacc.Bacc(
        "TRN2", target_bir_lowering=False, debug=False, num_devices=N_CORES
    )

    xa_d = nc.dram_tensor("xa", [DH + 1, H * W], F16, kind="ExternalInput")
    xb_d = nc.dram_tensor("xb", [DH + 1, H * W], F16, kind="ExternalInput")
    sa_d = nc.dram_tensor("sa", [DH + 1, COUT * DH], F16, kind="ExternalInput")
    sb_d = nc.dram_tensor("sb", [DH + 1, COUT * DH], F16, kind="ExternalInput")
    bias_d = nc.dram_tensor("biasb", [128, 32 * COUT], F32, kind="ExternalInput")
    out_d = nc.dram_tensor("out", [128, 32 * COUT], F32, kind="ExternalOutput")

    NTREE = 4 - DIRECT_CHUNKS          # chunks through the ACT+tree path
    TU = 16 * NTREE                    # tree units per row-pair

    with TileContext(nc) as tc:
        with (
            tc.tile_pool(name="io", bufs=1) as io_pool,
            tc.tile_pool(name="ps", bufs=2, space="PSUM") as ps_pool,
            tc.tile_pool(name="st", bufs=2) as st_pool,
            tc.tile_pool(name="tr", bufs=2) as tr_pool,
        ):
            xa_t = io_pool.tile([DH + 1, H * W], F16)
            nc.sync.dma_start(out=xa_t[:, :], in_=xa_d.ap())
            xb_t = io_pool.tile([DH + 1, H * W], F16)
            nc.sync.dma_start(out=xb_t[:, :], in_=xb_d.ap())
            sa_t = io_pool.tile([DH + 1, COUT * DH], F16)
            nc.sync.dma_start(out=sa_t[:, :], in_=sa_d.ap())
            sb_t = io_pool.tile([DH + 1, COUT * DH], F16)
            nc.sync.dma_start(out=sb_t[:, :], in_=sb_d.ap())
            bias_t = io_pool.tile([128, 32 * COUT], F32)
            nc.sync.dma_start(out=bias_t[:, :], in_=bias_d.ap())

            # acc columns: (r2, half, co)
            acc_t = io_pool.tile([128, 32 * 2 * COUT], F32)
            fin_t = io_pool.tile([128, 32 * COUT], F32)

            xa4 = xa_t[:, :].rearrange("k (h w) -> k h w", h=H)
            xb4 = xb_t[:, :].rearrange("k (h w) -> k h w", h=H)

            for r2 in range(32):
                r = 2 * r2
                staged = (
                    st_pool.tile([128, TU * DH], F16, tag="stg", name=f"stg{r2}")
                    if NTREE
                    else None
                )
                n_staged = 0
                for c in range(4):
                    half = 0 if c < 2 else 1
                    x_t = (xa4 if half == 0 else xb4)[:, r : r + 2, :]  # [73, 2, 64]
                    s_t = sa_t if half == 0 else sb_t
                    col0 = (c % 2) * 16 * DH
                    ps_t = ps_pool.tile([128, 16 * DH], F32, tag="ps")
                    for m0 in range(0, 16 * DH, 512):
                        m1 = min(m0 + 512, 16 * DH)
                        nc.tensor.matmul(
                            out=ps_t[:, m0:m1],
                            lhsT=x_t,
                            rhs=s_t[:, col0 + m0 : col0 + m1],
                            start=True,
                            stop=True,
                        )
                    if c < DIRECT_CHUNKS:
                        nc.vector.tensor_reduce(
                            out=acc_t[:, r2 * 64 + c * 16 : r2 * 64 + (c + 1) * 16],
                            in_=ps_t[:, :].rearrange("p (u j) -> p u j", j=DH),
                            axis=mybir.AxisListType.X,
                            op=AluOpType.max,
                            apply_absolute_value=True,
                        )
                    else:
                        nc.scalar.activation(
                            out=staged[:, n_staged * 16 * DH : (n_staged + 1) * 16 * DH],
                            in_=ps_t[:, :],
                            func=mybir.ActivationFunctionType.Abs,
                        )
                        n_staged += 1

                if NTREE:
                    s3 = staged[:, :].rearrange("p (u j) -> p u j", j=DH)
                    t1_t = tr_pool.tile([128, TU * 36], F16, tag="t1")
                    t1 = t1_t[:, :].rearrange("p (u j) -> p u j", j=36)
                    nc.vector.tensor_tensor(
                        out=t1, in0=s3[:, :, 0:36], in1=s3[:, :, 36:72],
                        op=AluOpType.max,
                    )
                    t2_t = tr_pool.tile([128, TU * 18], F16, tag="t2")
                    t2 = t2_t[:, :].rearrange("p (u j) -> p u j", j=18)
                    nc.vector.tensor_tensor(
                        out=t2, in0=t1[:, :, 0:18], in1=t1[:, :, 18:36],
                        op=AluOpType.max,
                    )
                    t3_t = tr_pool.tile([128, TU * 9], F16, tag="t3")
                    t3 = t3_t[:, :].rearrange("p (u j) -> p u j", j=9)
                    nc.vector.tensor_tensor(
                        out=t3, in0=t2[:, :, 0:9], in1=t2[:, :, 9:18],
                        op=AluOpType.max,
                    )
                    nc.vector.tensor_reduce(
                        out=acc_t[:, r2 * 64 + DIRECT_CHUNKS * 16 : r2 * 64 + 64],
                        in_=t3,
                        axis=mybir.AxisListType.X,
                        op=AluOpType.max,
                    )

            # combine halves: final[p, (r2, co)] = max(accA, accB) + bias
            a4 = acc_t[:, :].rearrange("p (r2 h c) -> p r2 h c", h=2, c=COUT)
            f3 = fin_t[:, :].rearrange("p (r2 c) -> p r2 c", c=COUT)
            nc.vector.tensor_tensor(
                out=f3, in0=a4[:, :, 0, :], in1=a4[:, :, 1, :], op=AluOpType.max
            )
            nc.vector.tensor_tensor(
                out=fin_t[:, :], in0=fin_t[:, :], in1=bias_t[:, :], op=AluOpType.add
            )
            nc.sync.dma_start(out=out_d.ap(), in_=fin_t[:, :])

    nc.compile()
    return nc


def _prep_inputs_pe(x, weights, bias):
    # shifted planes, d-order = (kw, kh, cin): d = kw*48 + kh*16 + cin
    # plane_d[h', w] = x[cin, clamp(h'+kh-1), clamp(w+kw-1)]
    in_maps = []
    w_perm = np.ascontiguousarray(weights.transpose(0, 3, 2, 1)).reshape(COUT, D)

    def selector(half):
        s = np.zeros((DH + 1, COUT, DH), dtype=np.float32)
        for j in range(DH):
            s[j, :, j] = 1.0
        s[DH, :, :] = -w_perm[:, half * DH : (half + 1) * DH]
        return s.reshape(DH + 1, COUT * DH).astype(np.float16)

    sa = np.ascontiguousarray(selector(0))
    sb = np.ascontiguousarray(selector(1))
    biasb = np.ascontiguousarray(
        np.broadcast_to(
            np.tile(bias.reshape(COUT), 32)[None, :], (128, 32 * COUT)
        ).astype(np.float32)
    )

    for core in range(N_CORES):
        xc = x[core]  # (CIN, H, W)
        x_pad = np.pad(xc, ((0, 0), (1, 1), (1, 1)), mode="edge")  # (CIN, 66, 66)
        planes = np.empty((3, 3, CIN, H, W), dtype=np.float32)  # (kw, kh, cin, h, w)
        for kw in range(3):
            for kh in range(3):
                planes[kw, kh] = x_pad[:, kh : kh + H, kw : kw + W]
        planes = planes.reshape(D, H * W)
        ones = np.ones((1, H * W), dtype=np.float32)
        xa = np.concatenate([planes[:DH], ones], axis=0).astype(np.float16)
        xb = np.concatenate([planes[DH:], ones], axis=0).astype(np.float16)
        in_maps.append(
            {
                "xa": np.ascontiguousarray(xa),
                "xb": np.ascontiguousarray(xb),
                "sa": sa,
                "sb": sb,
                "biasb": biasb,
            }
        )
    return in_maps


# ---------------------------------------------------------------- DVE scheme

def _build_program_dve():
    import concourse.bacc as bacc
    import concourse.mybir as mybir
    from concourse.alu_op_type import AluOpType
    from concourse.tile import TileContext

    dt = mybir.dt.float32 if COMPUTE == "f32" else mybir.dt.float16

    nc = bacc.Bacc(
        "TRN2", target_bir_lowering=False, debug=False, num_devices=N_CORES
    )

    x3b_d = nc.dram_tensor("x3b", [128, 3 * HPAD * CIN], dt, kind="ExternalInput")
    wb_d = nc.dram_tensor("wb", [128, COUT * D], dt, kind="ExternalInput")
    bias_d = nc.dram_tensor(
        "biasb", [128, 32 * COUT], mybir.dt.float32, kind="ExternalInput"
    )
    out_d = nc.dram_tensor(
        "out", [128, 32 * COUT], mybir.dt.float32, kind="ExternalOutput"
    )

    with TileContext(nc) as tc:
        with (
            tc.tile_pool(name="io", bufs=1) as io_pool,
            tc.tile_pool(name="sc", bufs=3) as spool,
        ):
            x3b_t = io_pool.tile([128, 3 * HPAD * CIN], dt)
            nc.sync.dma_start(out=x3b_t[:, :], in_=x3b_d.ap())
            wb_t = io_pool.tile([128, COUT * D], dt)
            nc.sync.dma_start(out=wb_t[:, :], in_=wb_d.ap())
            bias_t = io_pool.tile([128, 32 * COUT], mybir.dt.float32)
            nc.sync.dma_start(out=bias_t[:, :], in_=bias_d.ap())
            acc_t = io_pool.tile([128, 32 * COUT], mybir.dt.float32)

            x4 = x3b_t[:, :].rearrange("p (kw h c) -> p kw h c", kw=3, h=HPAD)
            w5 = wb_t[:, :].rearrange(
                "p (co kw kh c) -> p co kw kh c", co=COUT, kw=3, kh=3
            )

            for r2 in range(32):
                r = 2 * r2
                sc_t = spool.tile([128, COUT * D], dt, tag="sc")
                s5 = sc_t[:, :].rearrange(
                    "p (co kw kh c) -> p co kw kh c", co=COUT, kw=3, kh=3
                )
                x5b = (
                    x4[:, :, r : r + 3, :]
                    .unsqueeze(1)
                    .broadcast_to((128, COUT, 3, 3, CIN))
                )
                nc.vector.tensor_tensor(out=s5, in0=x5b, in1=w5, op=AluOpType.subtract)
                s3 = sc_t[:, :].rearrange("p (co d) -> p co d", co=COUT)
                nc.vector.tensor_reduce(
                    out=acc_t[:, r2 * COUT : (r2 + 1) * COUT],
                    in_=s3,
                    axis=mybir.AxisListType.X,
                    op=AluOpType.max,
                    apply_absolute_value=True,
                )

            nc.vector.tensor_tensor(
                out=acc_t[:, :], in0=acc_t[:, :], in1=bias_t[:, :], op=AluOpType.add
            )
            nc.sync.dma_start(out=out_d.ap(), in_=acc_t[:, :])

    nc.compile()
    return nc


def _np_dtype():
    return np.float32 if COMPUTE == "f32" else np.float16


def _build_x3b(xc):
    """xc: (CIN, H, W) float32 -> (128, 3*HPAD*CIN) in layout [(b,w), (kw, h_pad, cin)]."""
    dtype = _np_dtype()
    wi = np.clip(np.arange(W)[None, :] + np.arange(-1, 2)[:, None], 0, W - 1)  # (3, W)
    halves = []
    for b in range(2):
        h_idx = np.clip(np.arange(HPAD) - 1 + b, 0, H - 1)  # (HPAD,)
        g = xc[:, h_idx, :][:, :, wi]  # (CIN, HPAD, 3, W)
        halves.append(np.ascontiguousarray(g.transpose(3, 2, 1, 0)))
    out = np.stack(halves, axis=0)  # (2, W, 3, HPAD, CIN)
    return np.ascontiguousarray(out.reshape(128, 3 * HPAD * CIN).astype(dtype))


def _prep_inputs_dve(x, weights, bias):
    dtype = _np_dtype()
    wflat = np.ascontiguousarray(weights.transpose(0, 3, 2, 1)).reshape(1, COUT * D)
    wb = np.ascontiguousarray(np.broadcast_to(wflat, (128, COUT * D)).astype(dtype))
    biasb = np.ascontiguousarray(
        np.broadcast_to(
            np.tile(bias.reshape(COUT), 32)[None, :], (128, 32 * COUT)
        ).astype(np.float32)
    )
    return [
        {"x3b": _build_x3b(x[core]), "wb": wb, "biasb": biasb}
        for core in range(N_CORES)
    ]


# ---------------------------------------------------------------- common

def _get_program():
    key = (SCHEME, COMPUTE, DIRECT_CHUNKS, NB, ND)
    if key not in _PROGRAM_CACHE:
        if SCHEME == "hybrid":
            _PROGRAM_CACHE[key] = _build_program_hybrid()
        elif SCHEME == "pe":
            _PROGRAM_CACHE[key] = _build_program_pe()
        else:
            _PROGRAM_CACHE[key] = _build_program_dve()
    return _PROGRAM_CACHE[key]


def _prep_inputs(x, weights, bias):
    if SCHEME == "hybrid":
        return _prep_inputs_hybrid(x, weights, bias)
    if SCHEME == "pe":
        return _prep_inputs_pe(x, weights, bias)
    return _prep_inputs_dve(x, weights, bias)


def _unshuffle(o):
    """o: (128, 1024) [(b,w), (r2,co)] -> (COUT, H, W)"""
    return np.ascontiguousarray(
        np.asarray(o).reshape(2, W, 32, COUT).transpose(3, 2, 0, 1).reshape(COUT, H, W)
    )


def kernel(x, weights, bias):
    from concourse.bass_utils import run_bass_kernel_spmd

    global LAST_RESULTS
    nc = _get_program()

    x = np.asarray(x, dtype=np.float32)
    weights = np.asarray(weights, dtype=np.float32)
    bias = np.asarray(bias, dtype=np.float32)

    in_maps = _prep_inputs(x, weights, bias)
    res = run_bass_kernel_spmd(nc, in_maps, core_ids=list(range(N_CORES)))
    LAST_RESULTS = res

    outs = [_unshuffle(res.results[core]["out"]) for core in range(N_CORES)]
    return np.stack(outs).astype(np.float32)
